# revision 9
# baseline (speedup 1.0000x reference)
"""Multi-head causal attention with RoPE on 8 Trainium2 NeuronCores.

Problem: B=2, S=2048, D=1024, H=16 heads (dk=64), fp32 reference, causal
mask, RoPE on Q/K, y = softmax(QK^T/sqrt(dk)) V projected by Wo.

Head-parallel sharding: core c owns 2 heads (columns c*128:(c+1)*128 of
the QKV projection output). All matmul operands are bf16 (inputs rounded
on host; PSUM accumulation stays fp32), which halves HBM traffic and
weight-load time and runs every matmul at 1 cycle/row.

Per core:
  1. QKV projections from the full x, K-dim 1024, in 512-token chunks.
     Q/K come out transposed [feat, tok]; V in [tok, feat] for the PV
     matmul. Q/K feature order is pair-permuted on the host (evens then
     odds per head) so the RoPE partner swap is two contiguous 32-row
     block copies instead of a stride-2 shuffle; scores are invariant to
     the permutation.
  2. RoPE via cos/sin tables: PSUM->SBUF cast on ScalarE, block-swap DMA,
     two muls + add on DVE (bf16, 2x mode).
  3. Causal attention with transposed scores ST[k,q]: exp on ScalarE
     straight out of PSUM, causal-mask multiply on DVE restricted to the
     one 128-col triangle block per diagonal tile, and above-diagonal
     columns of diagonal tiles trimmed from the score/exp/PV ranges.
     A ones column appended to V makes the PV matmul emit the softmax
     denominator; 1/l via a single-pass approx reciprocal, broadcast
     across partitions with a K=1 ones matmul.
  4. Per-batch AllToAll (0.5 MB bf16) flips head-sharded -> token-sharded
     as soon as that batch's attention is done; batch 0's collective and
     output projection overlap batch 1's attention.
  5. Output rows are sharded per (batch, core): core c owns 256-row
     slices of both batches, so only batch 1's small collective+
     projection are exposed at the tail.

The emission order software-pipelines everything: each attention block
is interleaved instruction-by-instruction with the next projection chunk
(or with the previous batch's output projection) so the PE never idles
long enough to drop out of its fast p-state. A tiny dummy AllToAll at
kernel start absorbs the one-time collective warmup cost.
"""

import sys

for p in ("/opt/trn_rl_repo", "/root/.axon_site/_ro/trn_rl_repo"):
    if p not in sys.path:
        sys.path.insert(0, p)

import math

import ml_dtypes
import numpy as np

import concourse.bass as bass
import concourse.tile as tile
from concourse import mybir
from concourse.bass_utils import run_bass_kernel_spmd

N_CORES = 8
B, S, D, H = 2, 2048, 1024, 16
DK = D // H          # 64
HPC = H // N_CORES   # heads per core = 2
FW = HPC * DK        # head-group width per core = 128
T = B * S            # 4096 flattened tokens
TCH = 512            # token chunk for projections
NCH = T // TCH       # 8 chunks
KT = 128             # k tile
QC = 512             # q chunk in attention
OUTB = 256           # output token rows per core per batch

F32 = mybir.dt.float32
BF16 = mybir.dt.bfloat16
EXPF = mybir.ActivationFunctionType.Exp

BF = ml_dtypes.bfloat16


def _spill_waits(nc, max_other=1):
    """walrus in this container allows 1 sync-wait per instruction; move
    excess waits onto preceding single-wait NoOps on the same engine."""
    n_new = 0
    for bb in nc.m.functions[0].blocks:
        newlist = []
        changed = False
        for inst in bb.instructions:
            si = inst.sync_info
            if si is not None and si.on_wait and len(si.on_wait) > max_other:
                waits = list(si.on_wait)
                overflow, keep = waits[:-max_other], waits[-max_other:]
                while overflow:
                    chunk, overflow = overflow[:1], overflow[1:]
                    nop = mybir.InstNoOp(
                        name=f"waitspill{n_new}-{inst.name}", ins=[], outs=[]
                    )
                    nop.engine = inst.engine
                    nop.debug = inst.debug
                    nop.sync_info = mybir.SyncInfo(on_wait=chunk, on_update=[])
                    newlist.append(nop)
                    n_new += 1
                si.on_wait = keep
                inst.sync_info = si
                changed = True
            newlist.append(inst)
        if changed:
            bb.instructions = newlist
    return n_new


def build_kernel():
    nc = bass.Bass("TRN2", num_devices=N_CORES)

    xT3 = nc.dram_tensor("xT3", [128, NCH, 8, TCH], BF16, kind="ExternalInput")
    wq = nc.dram_tensor("wq", [128, 8, FW], BF16, kind="ExternalInput")
    wk = nc.dram_tensor("wk", [128, 8, FW], BF16, kind="ExternalInput")
    wv = nc.dram_tensor("wv", [128, 8, FW], BF16, kind="ExternalInput")
    wo = nc.dram_tensor("wo", [N_CORES, 128, D], BF16, kind="ExternalInput")
    ctab = nc.dram_tensor("ctab", [FW, S], BF16, kind="ExternalInput")
    stab = nc.dram_tensor("stab", [FW, S], BF16, kind="ExternalInput")
    trid = nc.dram_tensor("trid", [KT, KT], BF16, kind="ExternalInput")
    y = nc.dram_tensor("y", [B * OUTB, D], F32, kind="ExternalOutput")

    with tile.TileContext(nc) as tc:
        with (
            tc.tile_pool(name="const", bufs=1) as const,
            tc.tile_pool(name="xch", bufs=3) as xch,
            tc.tile_pool(name="qk", bufs=1) as qkpool,
            tc.tile_pool(name="tmp", bufs=2) as tmp,
            tc.tile_pool(name="pts", bufs=6) as pts,
            tc.tile_pool(name="lpool", bufs=2) as lpool,
            tc.tile_pool(name="wop", bufs=8) as wopool,
            tc.tile_pool(name="yout", bufs=2) as ypool,
            tc.tile_pool(name="orcv", bufs=2) as opool,
            tc.tile_pool(name="mm", bufs=3, space="PSUM") as mmps,
            tc.tile_pool(name="st", bufs=3, space="PSUM") as stps,
            tc.tile_pool(name="pv", bufs=2, space="PSUM") as pvps,
            tc.tile_pool(name="dram", bufs=1, space="DRAM") as dram,
        ):
            # ---- collective warmup (hidden under projection phase) ----
            warm_in = dram.tile([8, 16], F32, name="warm_in")
            warm_out = dram.tile([8, 16], F32, name="warm_out")
            wtile = const.tile([1, 128], F32, name="wtile")
            nc.vector.memset(wtile, 0.0)
            nc.gpsimd.dma_start(
                out=warm_in[:, :],
                in_=wtile[:1, :128].rearrange("p (a f) -> (p a) f", a=8),
            )
            nc.gpsimd.collective_compute(
                "AllToAll",
                mybir.AluOpType.bypass,
                replica_groups=[list(range(N_CORES))],
                ins=[warm_in[:].opt()],
                outs=[warm_out[:].opt()],
            )

            # ---- constants (wq/wk first so chunk0's matmuls start ASAP;
            # the rest loads while chunk0's x DMA streams) ----
            wq_sb = const.tile([128, 8, FW], BF16, name="wq_sb")
            wk_sb = const.tile([128, 8, FW], BF16, name="wk_sb")
            wv_sb = const.tile([128, 8, FW], BF16, name="wv_sb")
            nc.sync.dma_start(out=wq_sb, in_=wq[:, :, :])
            nc.sync.dma_start(out=wk_sb, in_=wk[:, :, :])
            c_sb = const.tile([FW, S], BF16, name="c_sb")
            s_sb = const.tile([FW, S], BF16, name="s_sb")
            tri_sb = const.tile([KT, KT], BF16, name="tri_sb")

            def late_consts():
                nc.sync.dma_start(out=wv_sb, in_=wv[:, :, :])
                nc.sync.dma_start(out=c_sb, in_=ctab[:, :])
                nc.sync.dma_start(out=s_sb, in_=stab[:, :])
                nc.sync.dma_start(out=tri_sb, in_=trid[:, :])
                nc.vector.memset(ones_f, 1.0)
                nc.vector.tensor_copy(out=ones64, in_=ones_f)
                nc.vector.memset(vones, 1.0)
                nc.vector.tensor_copy(out=v_sb[:, :, DK], in_=vones)
                nc.vector.tensor_copy(out=v_sb[:, :, 2 * DK + 1], in_=vones)

            ones_f = const.tile([1, DK], F32, name="ones_f")
            ones64 = const.tile([1, DK], BF16, name="ones64")
            qT = qkpool.tile([FW, T], BF16, tag="qT", name="qT")
            kTt = qkpool.tile([FW, T], BF16, tag="kT", name="kTt")
            v_sb = qkpool.tile([128, T // 128, 2 * DK + 2], BF16, tag="v", name="v_sb")
            outT = qkpool.tile([FW, T], BF16, tag="outT", name="outT")
            vones = const.tile([128, T // 128], F32, name="vones")

            wo_sb = []

            # ---- QKV projections + RoPE, one 512-token chunk ----
            def proj_chunk_steps(ci):
                t0 = ci * TCH
                sc = (ci % (S // TCH)) * TCH  # position within batch
                xc = xch.tile([128, 8, TCH], BF16, tag="x", name="xc")
                nc.sync.dma_start(out=xc, in_=xT3[:, ci, :, :])
                yield
                for which, w_sb, dst in (("q", wq_sb, qT), ("k", wk_sb, kTt)):
                    ps = mmps.tile([FW, TCH], F32, tag="mm", name=f"{which}ps")
                    for dt in range(8):
                        nc.tensor.matmul(
                            ps,
                            w_sb[:, dt, :],
                            xc[:, dt, :],
                            start=(dt == 0),
                            stop=(dt == 7),
                            skip_group_check=True,
                        )
                        if dt == 3:
                            yield
                    yield
                    raw = tmp.tile([FW, TCH], BF16, tag="raw", name="raw")
                    nc.vector.tensor_copy(out=raw, in_=ps)
                    swp = tmp.tile([FW, TCH], BF16, tag="swp", name="swp")
                    # RoPE partner swap: evens<->odds as 32-row block copies
                    for blk in range(4):
                        sb = (blk ^ 1) * 32
                        nc.sync.dma_start(
                            out=swp[blk * 32 : (blk + 1) * 32, :],
                            in_=raw[sb : sb + 32, :],
                        )
                    dslice = dst[:, t0 : t0 + TCH]
                    nc.vector.tensor_mul(dslice, raw, c_sb[:, sc : sc + TCH])
                    t2 = tmp.tile([FW, TCH], BF16, tag="t2", name="t2")
                    nc.vector.tensor_mul(t2, swp, s_sb[:, sc : sc + TCH])
                    nc.vector.tensor_add(dslice, dslice, t2)
                    yield
                for sub in range(TCH // 128):
                    vps = mmps.tile([128, FW], F32, tag="mm", name="vps")
                    for dt in range(8):
                        nc.tensor.matmul(
                            vps,
                            xc[:, dt, sub * 128 : (sub + 1) * 128],
                            wv_sb[:, dt, :],
                            start=(dt == 0),
                            stop=(dt == 7),
                            skip_group_check=True,
                        )
                    idx = t0 // 128 + sub
                    nc.vector.tensor_copy(out=v_sb[:, idx, 0:DK], in_=vps[:, 0:DK])
                    nc.vector.tensor_copy(
                        out=v_sb[:, idx, DK + 1 : 2 * DK + 1], in_=vps[:, DK : 2 * DK]
                    )
                    yield

            # ---- causal attention, one (batch, q-chunk) block ----
            # Transposed-scores flash style, pipelined one k-tile ahead.
            # Diagonal k-tiles trim their fully-masked leading columns.
            def attn_block_steps(b, qc):
                trow = b * S + qc * QC
                nkt = 4 * (qc + 1)
                pv2 = [
                    pvps.tile([DK + 1, QC], F32, tag="pv", name=f"pv{h2}")
                    for h2 in range(HPC)
                ]

                def emit_st(kt):
                    d = kt - 4 * qc
                    c0 = KT * d if d > 0 else 0
                    kcol = b * S + kt * KT
                    pair = []
                    for h2 in range(HPC):
                        fb = h2 * DK
                        st = stps.tile([KT, QC], F32, tag="st", name=f"st{h2}")
                        nc.tensor.matmul(
                            st[:, c0:QC],
                            kTt[fb : fb + DK, kcol : kcol + KT],
                            qT[fb : fb + DK, trow + c0 : trow + QC],
                            start=True,
                            stop=True,
                            skip_group_check=True,
                        )
                        pt = pts.tile([KT, QC], BF16, tag="pt", name=f"pt{h2}")
                        nc.scalar.activation(
                            out=pt[:, c0:QC], in_=st[:, c0:QC], func=EXPF
                        )
                        if d >= 0:
                            nc.vector.tensor_mul(
                                pt[:, c0 : c0 + KT], pt[:, c0 : c0 + KT], tri_sb
                            )
                        pair.append((pt, c0))
                    return pair

                def emit_pv(kt, pair):
                    for h2 in range(HPC):
                        pt, c0 = pair[h2]
                        vcol = h2 * (DK + 1)
                        nc.tensor.matmul(
                            pv2[h2][:, c0:QC],
                            v_sb[:, b * (S // 128) + kt, vcol : vcol + DK + 1],
                            pt[:, c0:QC],
                            start=(kt == 0),
                            stop=(kt == nkt - 1),
                            skip_group_check=True,
                        )

                prev = emit_st(0)
                yield
                for kt in range(1, nkt):
                    cur = emit_st(kt)
                    yield
                    emit_pv(kt - 1, prev)
                    prev = cur
                    yield
                emit_pv(nkt - 1, prev)
                yield
                for h2 in range(HPC):
                    fb = h2 * DK
                    pv = pv2[h2]
                    # 1/l = exp(-ln l) on ScalarE: walrus lowers the DVE
                    # reciprocal to ~6 passes (3.3us), this is 2x ~0.6us
                    lnl = lpool.tile([1, QC], F32, tag="linv", name="lnl")
                    nc.scalar.activation(
                        out=lnl, in_=pv[DK : DK + 1, :],
                        func=mybir.ActivationFunctionType.Ln,
                    )
                    linv_b = lpool.tile([1, QC], BF16, tag="linvb", name="linv_b")
                    nc.scalar.activation(
                        out=linv_b, in_=lnl,
                        func=EXPF, scale=-1.0,
                    )
                    # broadcast 1/l across the 64 head-dim partitions via a
                    # K=1 ones matmul (engines can't partition-broadcast)
                    lbps = stps.tile([DK, QC], F32, tag="st", name="lbps")
                    nc.tensor.matmul(
                        lbps, ones64, linv_b, start=True, stop=True,
                        skip_group_check=True,
                    )
                    # DVE can read only one PSUM operand per instruction
                    lb = lpool.tile([DK, QC], F32, tag="lb", name="lb")
                    nc.vector.tensor_copy(out=lb, in_=lbps)
                    nc.vector.tensor_mul(
                        outT[fb : fb + DK, trow : trow + QC], pv[0:DK, :], lb
                    )
                yield

            # ---- AllToAll staging/firing: head-sharded -> token-sharded ----
            def fire_cc(cc_in, cc_out, ntok, name):
                nc.gpsimd.collective_compute(
                    "AllToAll",
                    mybir.AluOpType.bypass,
                    replica_groups=[list(range(N_CORES))],
                    ins=[cc_in[:].opt()],
                    outs=[cc_out[:].opt()],
                )
                orecv = opool.tile(
                    [128, N_CORES, ntok], BF16, tag=name, bufs=1, name=name
                )
                for p in range(N_CORES):
                    nc.gpsimd.dma_start(out=orecv[:, p, :], in_=cc_out[p, :, :])
                return orecv

            # ---- output projection for a 128*k-token piece of this core's slice ----
            def outproj_steps(orecv, ntok, yrow):
                for tt in range(ntok // 128):
                    ysb = ypool.tile([128, D], F32, tag="y", name="ysb")
                    for ec in range(D // 512):
                        yps = mmps.tile([128, 512], F32, tag="mm", name="yps")
                        for p in range(N_CORES):
                            nc.tensor.matmul(
                                yps,
                                orecv[:, p, tt * 128 : (tt + 1) * 128],
                                wo_sb[p][:, ec * 512 : (ec + 1) * 512],
                                start=(p == 0),
                                stop=(p == N_CORES - 1),
                                skip_group_check=True,
                            )
                        nc.vector.tensor_copy(
                            out=ysb[:, ec * 512 : (ec + 1) * 512], in_=yps
                        )
                        yield
                    nc.sync.dma_start(
                        out=y[yrow + tt * 128 : yrow + (tt + 1) * 128, :],
                        in_=ysb,
                    )
                    yield

            def run_gen(g):
                for _ in g:
                    pass

            def interleave(ga, gb, na, nb):
                """Drain ga, stepping gb ~nb/na times per ga step; then drain gb."""
                ratio = na / nb
                acc = 0.0
                b_done = False
                for _ in ga:
                    acc += 1.0
                    while acc >= ratio and not b_done:
                        acc -= ratio
                        try:
                            next(gb)
                        except StopIteration:
                            b_done = True
                while not b_done:
                    try:
                        next(gb)
                    except StopIteration:
                        b_done = True

            PROJ_STEPS = 11

            # pipeline: chunk0, then attn(0,qc) x proj-chunk(qc+1), AllToAll(b0),
            # attn(1,qc) x proj-chunk(5+qc) with early per-block cc staging,
            # AllToAll(b1) in 2 halves, then outproj(b0) hiding the collective,
            # outproj(b1) halves as they land.
            g0 = proj_chunk_steps(0)
            next(g0)  # chunk0 x DMA issued
            late_consts()  # remaining const loads overlap it
            run_gen(g0)
            for qc in range(4):
                na = 2 * 4 * (qc + 1) + 1
                interleave(
                    attn_block_steps(0, qc), proj_chunk_steps(qc + 1), na, PROJ_STEPS
                )
            cc0_in = dram.tile([N_CORES, FW, OUTB], BF16, name="cc0in")
            cc0_out = dram.tile([N_CORES, FW, OUTB], BF16, name="cc0out")
            for p in range(N_CORES):
                nc.gpsimd.dma_start(
                    out=cc0_in[p, :, :], in_=outT[:, p * OUTB : (p + 1) * OUTB]
                )
            orecv0 = fire_cc(cc0_in, cc0_out, OUTB, "orecv0")
            for p in range(N_CORES):
                wt = wopool.tile([128, D], BF16, tag="wo", name=f"wo{p}")
                nc.sync.dma_start(out=wt, in_=wo[p, :, :])
                wo_sb.append(wt)

            cc1a_in = dram.tile([N_CORES, FW, 128], BF16, name="cc1ain")
            cc1a_out = dram.tile([N_CORES, FW, 128], BF16, name="cc1aout")
            cc1b_in = dram.tile([N_CORES, FW, 128], BF16, name="cc1bin")
            cc1b_out = dram.tile([N_CORES, FW, 128], BF16, name="cc1bout")

            def stage_cc1(qc):
                # after attn(1,qc), destination cores 2qc/2qc+1 slices are final
                for p in (2 * qc, 2 * qc + 1):
                    base = S + p * OUTB
                    nc.gpsimd.dma_start(
                        out=cc1a_in[p, :, :], in_=outT[:, base : base + 128]
                    )
                    nc.gpsimd.dma_start(
                        out=cc1b_in[p, :, :], in_=outT[:, base + 128 : base + 256]
                    )

            for qc in range(3):
                na = 2 * 4 * (qc + 1) + 1
                interleave(
                    attn_block_steps(1, qc), proj_chunk_steps(5 + qc), na, PROJ_STEPS
                )
                stage_cc1(qc)
            run_gen(attn_block_steps(1, 3))
            stage_cc1(3)
            orecv1a = fire_cc(cc1a_in, cc1a_out, 128, "orecv1a")
            orecv1b = fire_cc(cc1b_in, cc1b_out, 128, "orecv1b")
            # b0 projection runs while the b1 collectives fly
            run_gen(outproj_steps(orecv0, OUTB, 0))
            run_gen(outproj_steps(orecv1a, 128, OUTB))
            run_gen(outproj_steps(orecv1b, 128, OUTB + 128))

    _spill_waits(nc)
    return nc


_NC_CACHE = None


def _get_nc():
    global _NC_CACHE
    if _NC_CACHE is None:
        _NC_CACHE = build_kernel()
    return _NC_CACHE


def _host_prep(x, Wq, Wk, Wv, Wo, token_positions):
    # x -> [p, ci, dt, tc] so each partition's chunk slice is contiguous
    xT = x.reshape(T, D).T  # [D, T]
    xT3 = np.ascontiguousarray(
        xT.reshape(8, 128, NCH, TCH).transpose(1, 2, 0, 3)
    ).astype(BF)

    scale = np.float32(1.0 / math.sqrt(DK))
    WqT = Wq.T * scale  # [D, D], columns = output features
    WkT = np.ascontiguousarray(Wk.T)
    WvT = np.ascontiguousarray(Wv.T)
    WoT = np.ascontiguousarray(Wo.T)
    wo3 = WoT.reshape(N_CORES, 128, D).astype(BF)

    # per-head pair permutation (evens then odds) applied to q/k columns;
    # scores are invariant, and the RoPE partner swap becomes 32-row blocks
    pat = np.concatenate([np.arange(0, DK, 2), np.arange(1, DK, 2)])
    perm = np.concatenate([h * DK + pat for h in range(HPC)])

    pos = token_positions.astype(np.float64)  # [S]
    r = np.arange(FW)
    i = (r % DK) % 32  # pair index per row
    inv_freq = 1.0 / (10000.0 ** (2.0 * i / DK))  # [FW]
    ang = inv_freq[:, None] * pos[None, :]  # [FW, S]
    ctab = np.cos(ang).astype(BF)
    sgn = np.where((r % DK) < 32, -1.0, 1.0)
    stab = (np.sin(ang) * sgn[:, None]).astype(BF)

    tri = (np.arange(KT)[None, :] >= np.arange(KT)[:, None]).astype(BF)
    return xT3, WqT, WkT, WvT, wo3, perm, ctab, stab, tri


def kernel(x, Wq, Wk, Wv, Wo, mask, token_positions, num_heads, **run_kw):
    x = np.asarray(x)
    assert int(num_heads) == H and x.shape == (B, S, D)
    xT3, WqT, WkT, WvT, wo3, perm, ctab, stab, tri = _host_prep(
        np.asarray(x, np.float32),
        np.asarray(Wq, np.float32),
        np.asarray(Wk, np.float32),
        np.asarray(Wv, np.float32),
        np.asarray(Wo, np.float32),
        np.asarray(token_positions),
    )

    def wslice(WT, c, permute):
        cols = WT[:, c * FW : (c + 1) * FW]
        if permute:
            cols = cols[:, perm]
        # [D, FW] -> [p, dt, fw]
        return np.ascontiguousarray(
            cols.reshape(8, 128, FW).transpose(1, 0, 2)
        ).astype(BF)

    in_maps = []
    for c in range(N_CORES):
        in_maps.append(
            {
                "xT3": xT3,
                "wq": wslice(WqT, c, True),
                "wk": wslice(WkT, c, True),
                "wv": wslice(WvT, c, False),
                "wo": wo3,
                "ctab": ctab,
                "stab": stab,
                "trid": tri,
            }
        )
    nc = _get_nc()
    res = run_bass_kernel_spmd(
        nc, in_maps, core_ids=list(range(N_CORES)), **run_kw
    )
    yfull = np.empty((T, D), np.float32)
    for c in range(N_CORES):
        yc = np.asarray(res.results[c]["y"], np.float32)
        yfull[c * OUTB : (c + 1) * OUTB] = yc[0:OUTB]
        yfull[S + c * OUTB : S + (c + 1) * OUTB] = yc[OUTB : 2 * OUTB]
    out = yfull.reshape(B, S, D)
    kernel.last_results = res
    return out


# revision 10
# speedup vs baseline: 1.1138x; 1.1138x over previous
"""Multi-head causal attention with RoPE on 8 Trainium2 NeuronCores.

Problem: B=2, S=2048, D=1024, H=16 heads (dk=64), fp32 reference, causal
mask, RoPE on Q/K, y = softmax(QK^T/sqrt(dk)) V projected by Wo.

Head-parallel sharding: core c owns 2 heads (columns c*128:(c+1)*128 of
the QKV projection output). All matmul operands are bf16 (inputs rounded
on host; PSUM accumulation stays fp32), which halves HBM traffic and
weight-load time and runs every matmul at 1 cycle/row.

Per core:
  1. QKV projections from the full x, K-dim 1024, in 512-token chunks.
     Q/K come out transposed [feat, tok]; V in [tok, feat] for the PV
     matmul. Q/K feature order is pair-permuted on the host (evens then
     odds per head) so the RoPE partner swap is two contiguous 32-row
     block copies instead of a stride-2 shuffle; scores are invariant to
     the permutation.
  2. RoPE via cos/sin tables: PSUM->SBUF cast on ScalarE, block-swap DMA,
     two muls + add on DVE (bf16, 2x mode).
  3. Causal attention with transposed scores ST[k,q]: exp on ScalarE
     straight out of PSUM, causal-mask multiply on DVE restricted to the
     one 128-col triangle block per diagonal tile, and above-diagonal
     columns of diagonal tiles trimmed from the score/exp/PV ranges.
     A ones column appended to V makes the PV matmul emit the softmax
     denominator; 1/l via a single-pass approx reciprocal, broadcast
     across partitions with a K=1 ones matmul.
  4. Per-batch AllToAll (0.5 MB bf16) flips head-sharded -> token-sharded
     as soon as that batch's attention is done; batch 0's collective and
     output projection overlap batch 1's attention.
  5. Output rows are sharded per (batch, core): core c owns 256-row
     slices of both batches, so only batch 1's small collective+
     projection are exposed at the tail.

The emission order software-pipelines everything: each attention block
is interleaved instruction-by-instruction with the next projection chunk
(or with the previous batch's output projection) so the PE never idles
long enough to drop out of its fast p-state. A tiny dummy AllToAll at
kernel start absorbs the one-time collective warmup cost.
"""

import sys

for p in ("/opt/trn_rl_repo", "/root/.axon_site/_ro/trn_rl_repo"):
    if p not in sys.path:
        sys.path.insert(0, p)

import math

import ml_dtypes
import numpy as np

import concourse.bass as bass
import concourse.tile as tile
from concourse import mybir
from concourse.bass_utils import run_bass_kernel_spmd

N_CORES = 8
B, S, D, H = 2, 2048, 1024, 16
DK = D // H          # 64
HPC = H // N_CORES   # heads per core = 2
FW = HPC * DK        # head-group width per core = 128
T = B * S            # 4096 flattened tokens
TCH = 512            # token chunk for projections
NCH = T // TCH       # 8 chunks
KT = 128             # k tile
QC = 512             # q chunk in attention
OUTB = 256           # output token rows per core per batch

F32 = mybir.dt.float32
BF16 = mybir.dt.bfloat16
EXPF = mybir.ActivationFunctionType.Exp

BF = ml_dtypes.bfloat16


def _spill_waits(nc, max_other=1):
    """walrus in this container allows 1 sync-wait per instruction; move
    excess waits onto preceding single-wait NoOps on the same engine."""
    n_new = 0
    for bb in nc.m.functions[0].blocks:
        newlist = []
        changed = False
        for inst in bb.instructions:
            si = inst.sync_info
            if si is not None and si.on_wait and len(si.on_wait) > max_other:
                waits = list(si.on_wait)
                overflow, keep = waits[:-max_other], waits[-max_other:]
                while overflow:
                    chunk, overflow = overflow[:1], overflow[1:]
                    nop = mybir.InstNoOp(
                        name=f"waitspill{n_new}-{inst.name}", ins=[], outs=[]
                    )
                    nop.engine = inst.engine
                    nop.debug = inst.debug
                    nop.sync_info = mybir.SyncInfo(on_wait=chunk, on_update=[])
                    newlist.append(nop)
                    n_new += 1
                si.on_wait = keep
                inst.sync_info = si
                changed = True
            newlist.append(inst)
        if changed:
            bb.instructions = newlist
    return n_new


def build_kernel():
    nc = bass.Bass("TRN2", num_devices=N_CORES)

    xT3 = nc.dram_tensor("xT3", [128, NCH, 8, TCH], BF16, kind="ExternalInput")
    wq = nc.dram_tensor("wq", [128, 8, FW], BF16, kind="ExternalInput")
    wk = nc.dram_tensor("wk", [128, 8, FW], BF16, kind="ExternalInput")
    wv = nc.dram_tensor("wv", [128, 8, FW], BF16, kind="ExternalInput")
    wo = nc.dram_tensor("wo", [N_CORES, 128, D], BF16, kind="ExternalInput")
    ctab = nc.dram_tensor("ctab", [FW, S], BF16, kind="ExternalInput")
    stab = nc.dram_tensor("stab", [FW, S], BF16, kind="ExternalInput")
    trid = nc.dram_tensor("trid", [KT, KT], BF16, kind="ExternalInput")
    y = nc.dram_tensor("y", [B * OUTB, D], F32, kind="ExternalOutput")

    with tile.TileContext(nc) as tc:
        with (
            tc.tile_pool(name="const", bufs=1) as const,
            tc.tile_pool(name="xch", bufs=3) as xch,
            tc.tile_pool(name="qk", bufs=1) as qkpool,
            tc.tile_pool(name="tmp", bufs=2) as tmp,
            tc.tile_pool(name="pts", bufs=6) as pts,
            tc.tile_pool(name="lpool", bufs=2) as lpool,
            tc.tile_pool(name="wop", bufs=8) as wopool,
            tc.tile_pool(name="yout", bufs=2) as ypool,
            tc.tile_pool(name="orcv", bufs=2) as opool,
            tc.tile_pool(name="mm", bufs=3, space="PSUM") as mmps,
            tc.tile_pool(name="st", bufs=3, space="PSUM") as stps,
            tc.tile_pool(name="pv", bufs=2, space="PSUM") as pvps,
            tc.tile_pool(name="dram", bufs=1, space="DRAM") as dram,
        ):
            # ---- collective warmup (hidden under projection phase) ----
            warm_in = dram.tile([8, 16], F32, name="warm_in")
            warm_out = dram.tile([8, 16], F32, name="warm_out")
            wtile = const.tile([1, 128], F32, name="wtile")
            nc.vector.memset(wtile, 0.0)
            nc.gpsimd.dma_start(
                out=warm_in[:, :],
                in_=wtile[:1, :128].rearrange("p (a f) -> (p a) f", a=8),
            )
            nc.gpsimd.collective_compute(
                "AllToAll",
                mybir.AluOpType.bypass,
                replica_groups=[list(range(N_CORES))],
                ins=[warm_in[:].opt()],
                outs=[warm_out[:].opt()],
            )

            # ---- constants (wq/wk first so chunk0's matmuls start ASAP;
            # the rest loads while chunk0's x DMA streams) ----
            wq_sb = const.tile([128, 8, FW], BF16, name="wq_sb")
            wk_sb = const.tile([128, 8, FW], BF16, name="wk_sb")
            wv_sb = const.tile([128, 8, FW], BF16, name="wv_sb")
            nc.sync.dma_start(out=wq_sb, in_=wq[:, :, :])
            nc.sync.dma_start(out=wk_sb, in_=wk[:, :, :])
            c_sb = const.tile([FW, S], BF16, name="c_sb")
            s_sb = const.tile([FW, S], BF16, name="s_sb")
            tri_sb = const.tile([KT, KT], BF16, name="tri_sb")

            def late_consts():
                nc.sync.dma_start(out=wv_sb, in_=wv[:, :, :])
                nc.sync.dma_start(out=c_sb, in_=ctab[:, :])
                nc.sync.dma_start(out=s_sb, in_=stab[:, :])
                nc.sync.dma_start(out=tri_sb, in_=trid[:, :])
                nc.vector.memset(ones_f, 1.0)
                nc.vector.tensor_copy(out=ones64, in_=ones_f)
                nc.vector.memset(vones, 1.0)
                nc.vector.tensor_copy(out=v_sb[:, :, DK], in_=vones)
                nc.vector.tensor_copy(out=v_sb[:, :, 2 * DK + 1], in_=vones)

            ones_f = const.tile([1, DK], F32, name="ones_f")
            ones64 = const.tile([1, DK], BF16, name="ones64")
            qT = qkpool.tile([FW, T], BF16, tag="qT", name="qT")
            kTt = qkpool.tile([FW, T], BF16, tag="kT", name="kTt")
            v_sb = qkpool.tile([128, T // 128, 2 * DK + 2], BF16, tag="v", name="v_sb")
            outT = qkpool.tile([FW, T], BF16, tag="outT", name="outT")
            vones = const.tile([128, T // 128], F32, name="vones")

            wo_sb = []

            # ---- QKV projections + RoPE, one 512-token chunk ----
            def proj_chunk_steps(ci):
                t0 = ci * TCH
                sc = (ci % (S // TCH)) * TCH  # position within batch
                xc = xch.tile([128, 8, TCH], BF16, tag="x", name="xc")
                nc.sync.dma_start(out=xc, in_=xT3[:, ci, :, :])
                yield
                for which, w_sb, dst in (("q", wq_sb, qT), ("k", wk_sb, kTt)):
                    ps = mmps.tile([FW, TCH], F32, tag="mm", name=f"{which}ps")
                    for dt in range(8):
                        nc.tensor.matmul(
                            ps,
                            w_sb[:, dt, :],
                            xc[:, dt, :],
                            start=(dt == 0),
                            stop=(dt == 7),
                            skip_group_check=True,
                        )
                        if dt == 3:
                            yield
                    yield
                    raw = tmp.tile([FW, TCH], BF16, tag="raw", name="raw")
                    nc.scalar.copy(out=raw, in_=ps)
                    swp = tmp.tile([FW, TCH], BF16, tag="swp", name="swp")
                    # RoPE partner swap: evens<->odds as 32-row block copies
                    for blk in range(4):
                        sb = (blk ^ 1) * 32
                        nc.sync.dma_start(
                            out=swp[blk * 32 : (blk + 1) * 32, :],
                            in_=raw[sb : sb + 32, :],
                        )
                    dslice = dst[:, t0 : t0 + TCH]
                    nc.vector.tensor_mul(dslice, raw, c_sb[:, sc : sc + TCH])
                    t2 = tmp.tile([FW, TCH], BF16, tag="t2", name="t2")
                    nc.vector.tensor_mul(t2, swp, s_sb[:, sc : sc + TCH])
                    nc.vector.tensor_add(dslice, dslice, t2)
                    yield
                for sub in range(TCH // 128):
                    vps = mmps.tile([128, FW], F32, tag="mm", name="vps")
                    for dt in range(8):
                        nc.tensor.matmul(
                            vps,
                            xc[:, dt, sub * 128 : (sub + 1) * 128],
                            wv_sb[:, dt, :],
                            start=(dt == 0),
                            stop=(dt == 7),
                            skip_group_check=True,
                        )
                    idx = t0 // 128 + sub
                    nc.scalar.copy(out=v_sb[:, idx, 0:DK], in_=vps[:, 0:DK])
                    nc.scalar.copy(
                        out=v_sb[:, idx, DK + 1 : 2 * DK + 1], in_=vps[:, DK : 2 * DK]
                    )
                    yield

            # ---- causal attention, one (batch, q-chunk) block ----
            # Transposed-scores flash style, pipelined one k-tile ahead.
            # Diagonal k-tiles trim their fully-masked leading columns.
            def attn_block_steps(b, qc):
                trow = b * S + qc * QC
                nkt = 4 * (qc + 1)
                pv2 = [
                    pvps.tile([DK + 1, QC], F32, tag="pv", name=f"pv{h2}")
                    for h2 in range(HPC)
                ]

                def emit_st(kt):
                    d = kt - 4 * qc
                    c0 = KT * d if d > 0 else 0
                    kcol = b * S + kt * KT
                    pair = []
                    for h2 in range(HPC):
                        fb = h2 * DK
                        st = stps.tile([KT, QC], F32, tag="st", name=f"st{h2}")
                        nc.tensor.matmul(
                            st[:, c0:QC],
                            kTt[fb : fb + DK, kcol : kcol + KT],
                            qT[fb : fb + DK, trow + c0 : trow + QC],
                            start=True,
                            stop=True,
                            skip_group_check=True,
                        )
                        pt = pts.tile([KT, QC], BF16, tag="pt", name=f"pt{h2}")
                        nc.scalar.activation(
                            out=pt[:, c0:QC], in_=st[:, c0:QC], func=EXPF
                        )
                        if d >= 0:
                            nc.vector.tensor_mul(
                                pt[:, c0 : c0 + KT], pt[:, c0 : c0 + KT], tri_sb
                            )
                        pair.append((pt, c0))
                    return pair

                def emit_pv(kt, pair):
                    for h2 in range(HPC):
                        pt, c0 = pair[h2]
                        vcol = h2 * (DK + 1)
                        nc.tensor.matmul(
                            pv2[h2][:, c0:QC],
                            v_sb[:, b * (S // 128) + kt, vcol : vcol + DK + 1],
                            pt[:, c0:QC],
                            start=(kt == 0),
                            stop=(kt == nkt - 1),
                            skip_group_check=True,
                        )

                prev = emit_st(0)
                yield
                for kt in range(1, nkt):
                    cur = emit_st(kt)
                    yield
                    emit_pv(kt - 1, prev)
                    prev = cur
                    yield
                emit_pv(nkt - 1, prev)
                yield
                for h2 in range(HPC):
                    fb = h2 * DK
                    pv = pv2[h2]
                    # 1/l = exp(-ln l) on ScalarE: walrus lowers the DVE
                    # reciprocal to ~6 passes (3.3us), this is 2x ~0.6us
                    lnl = lpool.tile([1, QC], F32, tag="linv", name="lnl")
                    nc.scalar.activation(
                        out=lnl, in_=pv[DK : DK + 1, :],
                        func=mybir.ActivationFunctionType.Ln,
                    )
                    linv_b = lpool.tile([1, QC], BF16, tag="linvb", name="linv_b")
                    nc.scalar.activation(
                        out=linv_b, in_=lnl,
                        func=EXPF, scale=-1.0,
                    )
                    # broadcast 1/l across the 64 head-dim partitions via a
                    # K=1 ones matmul (engines can't partition-broadcast)
                    lbps = stps.tile([DK, QC], F32, tag="st", name="lbps")
                    nc.tensor.matmul(
                        lbps, ones64, linv_b, start=True, stop=True,
                        skip_group_check=True,
                    )
                    # DVE can read only one PSUM operand per instruction
                    lb = lpool.tile([DK, QC], F32, tag="lb", name="lb")
                    nc.vector.tensor_copy(out=lb, in_=lbps)
                    nc.vector.tensor_mul(
                        outT[fb : fb + DK, trow : trow + QC], pv[0:DK, :], lb
                    )
                yield

            # ---- AllToAll staging/firing: head-sharded -> token-sharded ----
            def fire_cc(cc_in, cc_out, ntok, name):
                nc.gpsimd.collective_compute(
                    "AllToAll",
                    mybir.AluOpType.bypass,
                    replica_groups=[list(range(N_CORES))],
                    ins=[cc_in[:].opt()],
                    outs=[cc_out[:].opt()],
                )
                orecv = opool.tile(
                    [128, N_CORES, ntok], BF16, tag=name, bufs=1, name=name
                )
                for p in range(N_CORES):
                    nc.gpsimd.dma_start(out=orecv[:, p, :], in_=cc_out[p, :, :])
                return orecv

            # ---- output projection for a 128*k-token piece of this core's slice ----
            def outproj_steps(orecv, ntok, yrow):
                for tt in range(ntok // 128):
                    ysb = ypool.tile([128, D], F32, tag="y", name="ysb")
                    for ec in range(D // 512):
                        yps = mmps.tile([128, 512], F32, tag="mm", name="yps")
                        for p in range(N_CORES):
                            nc.tensor.matmul(
                                yps,
                                orecv[:, p, tt * 128 : (tt + 1) * 128],
                                wo_sb[p][:, ec * 512 : (ec + 1) * 512],
                                start=(p == 0),
                                stop=(p == N_CORES - 1),
                                skip_group_check=True,
                            )
                        nc.vector.tensor_copy(
                            out=ysb[:, ec * 512 : (ec + 1) * 512], in_=yps
                        )
                        yield
                    nc.sync.dma_start(
                        out=y[yrow + tt * 128 : yrow + (tt + 1) * 128, :],
                        in_=ysb,
                    )
                    yield

            def run_gen(g):
                for _ in g:
                    pass

            def interleave(ga, gb, na, nb):
                """Drain ga, stepping gb ~nb/na times per ga step; then drain gb."""
                ratio = na / nb
                acc = 0.0
                b_done = False
                for _ in ga:
                    acc += 1.0
                    while acc >= ratio and not b_done:
                        acc -= ratio
                        try:
                            next(gb)
                        except StopIteration:
                            b_done = True
                while not b_done:
                    try:
                        next(gb)
                    except StopIteration:
                        b_done = True

            PROJ_STEPS = 11

            # pipeline: chunk0, then attn(0,qc) x proj-chunk(qc+1), AllToAll(b0),
            # attn(1,qc) x proj-chunk(5+qc) with early per-block cc staging,
            # AllToAll(b1) in 2 halves, then outproj(b0) hiding the collective,
            # outproj(b1) halves as they land.
            g0 = proj_chunk_steps(0)
            next(g0)  # chunk0 x DMA issued
            late_consts()  # remaining const loads overlap it
            run_gen(g0)
            for qc in range(4):
                na = 2 * 4 * (qc + 1) + 1
                interleave(
                    attn_block_steps(0, qc), proj_chunk_steps(qc + 1), na, PROJ_STEPS
                )
            cc0_in = dram.tile([N_CORES, FW, OUTB], BF16, name="cc0in")
            cc0_out = dram.tile([N_CORES, FW, OUTB], BF16, name="cc0out")
            for p in range(N_CORES):
                nc.gpsimd.dma_start(
                    out=cc0_in[p, :, :], in_=outT[:, p * OUTB : (p + 1) * OUTB]
                )
            orecv0 = fire_cc(cc0_in, cc0_out, OUTB, "orecv0")
            for p in range(N_CORES):
                wt = wopool.tile([128, D], BF16, tag="wo", name=f"wo{p}")
                nc.sync.dma_start(out=wt, in_=wo[p, :, :])
                wo_sb.append(wt)

            cc1a_in = dram.tile([N_CORES, FW, 128], BF16, name="cc1ain")
            cc1a_out = dram.tile([N_CORES, FW, 128], BF16, name="cc1aout")
            cc1b_in = dram.tile([N_CORES, FW, 128], BF16, name="cc1bin")
            cc1b_out = dram.tile([N_CORES, FW, 128], BF16, name="cc1bout")

            def stage_cc1(qc):
                # after attn(1,qc), destination cores 2qc/2qc+1 slices are final
                for p in (2 * qc, 2 * qc + 1):
                    base = S + p * OUTB
                    nc.gpsimd.dma_start(
                        out=cc1a_in[p, :, :], in_=outT[:, base : base + 128]
                    )
                    nc.gpsimd.dma_start(
                        out=cc1b_in[p, :, :], in_=outT[:, base + 128 : base + 256]
                    )

            for qc in range(3):
                na = 2 * 4 * (qc + 1) + 1
                interleave(
                    attn_block_steps(1, qc), proj_chunk_steps(5 + qc), na, PROJ_STEPS
                )
                stage_cc1(qc)
            run_gen(attn_block_steps(1, 3))
            stage_cc1(3)
            orecv1a = fire_cc(cc1a_in, cc1a_out, 128, "orecv1a")
            orecv1b = fire_cc(cc1b_in, cc1b_out, 128, "orecv1b")
            # b0 projection runs while the b1 collectives fly
            run_gen(outproj_steps(orecv0, OUTB, 0))
            run_gen(outproj_steps(orecv1a, 128, OUTB))
            run_gen(outproj_steps(orecv1b, 128, OUTB + 128))

    _spill_waits(nc)
    return nc


_NC_CACHE = None


def _get_nc():
    global _NC_CACHE
    if _NC_CACHE is None:
        _NC_CACHE = build_kernel()
    return _NC_CACHE


def _host_prep(x, Wq, Wk, Wv, Wo, token_positions):
    # x -> [p, ci, dt, tc] so each partition's chunk slice is contiguous
    xT = x.reshape(T, D).T  # [D, T]
    xT3 = np.ascontiguousarray(
        xT.reshape(8, 128, NCH, TCH).transpose(1, 2, 0, 3)
    ).astype(BF)

    scale = np.float32(1.0 / math.sqrt(DK))
    WqT = Wq.T * scale  # [D, D], columns = output features
    WkT = np.ascontiguousarray(Wk.T)
    WvT = np.ascontiguousarray(Wv.T)
    WoT = np.ascontiguousarray(Wo.T)
    wo3 = WoT.reshape(N_CORES, 128, D).astype(BF)

    # per-head pair permutation (evens then odds) applied to q/k columns;
    # scores are invariant, and the RoPE partner swap becomes 32-row blocks
    pat = np.concatenate([np.arange(0, DK, 2), np.arange(1, DK, 2)])
    perm = np.concatenate([h * DK + pat for h in range(HPC)])

    pos = token_positions.astype(np.float64)  # [S]
    r = np.arange(FW)
    i = (r % DK) % 32  # pair index per row
    inv_freq = 1.0 / (10000.0 ** (2.0 * i / DK))  # [FW]
    ang = inv_freq[:, None] * pos[None, :]  # [FW, S]
    ctab = np.cos(ang).astype(BF)
    sgn = np.where((r % DK) < 32, -1.0, 1.0)
    stab = (np.sin(ang) * sgn[:, None]).astype(BF)

    tri = (np.arange(KT)[None, :] >= np.arange(KT)[:, None]).astype(BF)
    return xT3, WqT, WkT, WvT, wo3, perm, ctab, stab, tri


def kernel(x, Wq, Wk, Wv, Wo, mask, token_positions, num_heads, **run_kw):
    x = np.asarray(x)
    assert int(num_heads) == H and x.shape == (B, S, D)
    xT3, WqT, WkT, WvT, wo3, perm, ctab, stab, tri = _host_prep(
        np.asarray(x, np.float32),
        np.asarray(Wq, np.float32),
        np.asarray(Wk, np.float32),
        np.asarray(Wv, np.float32),
        np.asarray(Wo, np.float32),
        np.asarray(token_positions),
    )

    def wslice(WT, c, permute):
        cols = WT[:, c * FW : (c + 1) * FW]
        if permute:
            cols = cols[:, perm]
        # [D, FW] -> [p, dt, fw]
        return np.ascontiguousarray(
            cols.reshape(8, 128, FW).transpose(1, 0, 2)
        ).astype(BF)

    in_maps = []
    for c in range(N_CORES):
        in_maps.append(
            {
                "xT3": xT3,
                "wq": wslice(WqT, c, True),
                "wk": wslice(WkT, c, True),
                "wv": wslice(WvT, c, False),
                "wo": wo3,
                "ctab": ctab,
                "stab": stab,
                "trid": tri,
            }
        )
    nc = _get_nc()
    res = run_bass_kernel_spmd(
        nc, in_maps, core_ids=list(range(N_CORES)), **run_kw
    )
    yfull = np.empty((T, D), np.float32)
    for c in range(N_CORES):
        yc = np.asarray(res.results[c]["y"], np.float32)
        yfull[c * OUTB : (c + 1) * OUTB] = yc[0:OUTB]
        yfull[S + c * OUTB : S + (c + 1) * OUTB] = yc[OUTB : 2 * OUTB]
    out = yfull.reshape(B, S, D)
    kernel.last_results = res
    return out
